# revision 3
# baseline (speedup 1.0000x reference)
"""GraphSAGE-mean 2-layer GNN on 8 Trainium2 NeuronCores (Bass/Tile).

Sharding: nodes split into 8 contiguous ranges (rows c*12500..): core c
computes output rows for its range.  The full feature table is replicated per
core; layer-1 results are AllGather'd to rebuild the replicated table for
layer 2.

Aggregation: per core, edges (grouped by dst) are split into 4 passes by src
chunk of 32768 rows so src indices fit the int16 index format of the custom
dma_gather ucode (4096 rows per instruction).  Segment-sum runs on the tensor
engine: for each 128-edge block a selection matrix
  sel[e, m] = (dstl[e] == m) * invdeg[dst[e]]
is built in one fused DVE op from a constant iota tile, and
  psum[f, m] += msgs[e, f]^T @ sel[e, m]
accumulates weighted neighbor sums for one 128-node tile, feature-major.
The self path is contiguous loads + PE transpose; the transform computes
out^T = W_neigh^T @ aggT + W_self^T @ selfT with bias+relu fused into one
ScalarE activation, then PE-transposes back to node-major rows.

The SPMD program is shared by all 8 cores, so per-(pass, tile) block counts
are static = max over the 8 cores; shorter cores pad with zero-weight slots.
"""

import numpy as np

N = 100000
F = 128
NCORES = 8
OWN = N // NCORES            # 12500
P = 128
NTILES = (OWN + P - 1) // P  # 98
OWN_PAD = NTILES * P         # 12544
N_PAD = 100096               # table rows padded to a multiple of 128
CHUNK = 32768
NPASS = (N + CHUNK - 1) // CHUNK  # 4
GBS = 1024                   # gather rows per dma_gather instruction (SWDGE ring holds 1024 descs)
BLK = 128                    # edges per block


# --------------------------------------------------------------------------
# host-side planning
# --------------------------------------------------------------------------

def _plan(edge_src, edge_dst):
    src = np.asarray(edge_src).astype(np.int64).ravel()
    dst = np.asarray(edge_dst).astype(np.int64).ravel()
    deg = np.bincount(dst, minlength=N)
    invdeg = (1.0 / np.maximum(deg, 1)).astype(np.float32)

    per_core = []
    owner = dst // OWN
    for c in range(NCORES):
        m = owner == c
        s, d = src[m], dst[m]
        p = s // CHUNK
        order = np.lexsort((d, p))
        per_core.append((s[order], d[order], p[order]))

    cnt = np.zeros((NCORES, NPASS, NTILES), dtype=np.int64)
    for c in range(NCORES):
        s, d, p = per_core[c]
        t = (d - c * OWN) // P
        np.add.at(cnt, (c, p, t), 1)
    B = np.ceil(cnt.max(axis=0) / BLK).astype(np.int64)   # [NPASS, NTILES]

    nblk_pass = B.sum(axis=1).astype(np.int64)
    nblk = int(nblk_pass.sum())
    blk_tile = np.concatenate(
        [np.repeat(np.arange(NTILES), B[p]) for p in range(NPASS)]
    ).astype(np.int64)

    plans = []
    for c in range(NCORES):
        s, d, p = per_core[c]
        idx16 = np.zeros(nblk * BLK, dtype=np.int16)
        dstl = np.full(nblk * BLK, -1.0, dtype=np.float32)
        w = np.zeros(nblk * BLK, dtype=np.float32)
        blk0 = 0
        for pp in range(NPASS):
            m = p == pp
            sp, dp = s[m], d[m]
            tp = (dp - c * OWN) // P
            for t in range(NTILES):
                bcount = int(B[pp, t])
                if bcount == 0:
                    continue
                em = tp == t
                se, de = sp[em], dp[em]
                ne = se.shape[0]
                assert ne <= bcount * BLK
                base = blk0 * BLK
                idx16[base : base + ne] = (se - pp * CHUNK).astype(np.int16)
                dstl[base : base + ne] = (de - c * OWN - t * P).astype(np.float32)
                w[base : base + ne] = invdeg[de]
                blk0 += bcount
        assert blk0 == nblk
        plans.append({"idx16": idx16, "dstl": dstl, "w": w})

    return plans, B, blk_tile, nblk_pass, nblk


def _gather_instruction_sizes(nblk_pass):
    """Mirror of the device loop: list of (pass, blocks) per gather inst."""
    out = []
    for pp in range(NPASS):
        nb = int(nblk_pass[pp])
        done = 0
        while done < nb:
            take = min(GBS // BLK, nb - done)
            out.append((pp, take))
            done += take
    return out


def _pack_gidx(idx16, nblk_pass):
    """Pack int16 indices in the dma_gather SBUF layout (position j ->
    partition j%16, column j//16, replicated to 128 partitions) as one
    [128, total_cols] plane with per-instruction column segments, raveled
    partition-major.  Loaded to SBUF once and sliced per instruction."""
    total_cols = sum(take * BLK // 16
                     for _pp, take in _gather_instruction_sizes(nblk_pass))
    out = np.zeros((128, total_cols), dtype=np.int16)
    cursor = 0
    col = 0
    for _pp, take in _gather_instruction_sizes(nblk_pass):
        rows = take * BLK
        seg = idx16[cursor : cursor + rows]
        cursor += rows
        w16 = seg.reshape(rows // 16, 16).T          # [16, cols]
        out[:, col : col + rows // 16] = np.tile(w16, (8, 1))
        col += rows // 16
    return out.ravel()


def _schedule_flags(B):
    """start/stop flags per block within each pass (blocks are emitted
    pass-major, grouped by tile)."""
    firsts, lasts = [], []
    for pp in range(NPASS):
        tiles = [int(t) for t in np.repeat(np.arange(NTILES), B[pp])]
        f = [i == 0 or tiles[i] != tiles[i - 1] for i in range(len(tiles))]
        l = [i + 1 == len(tiles) or tiles[i + 1] != tiles[i]
             for i in range(len(tiles))]
        firsts.append(f)
        lasts.append(l)
    return firsts, lasts


# --------------------------------------------------------------------------
# device program
# --------------------------------------------------------------------------

def _build(B, blk_tile, nblk_pass, nblk, skip_collective=False):
    import concourse.bass as bass
    import concourse.mybir as mybir
    import concourse.tile as tile
    from concourse import library_config
    from concourse.masks import make_identity
    from concourse.tile_rust import add_dep_helper

    nc = bass.Bass("TRN2", target_bir_lowering=False, debug=False,
                   num_devices=NCORES, num_swdge_queues=4)
    dt = mybir.dt

    x_rep = nc.dram_tensor("x_rep", [N_PAD, F], dt.float32, kind="ExternalInput")
    x_self = nc.dram_tensor("x_self", [OWN_PAD, F], dt.float32,
                            kind="ExternalInput")
    gidx_len = sum(128 * (take * BLK // 16)
                   for _pp, take in _gather_instruction_sizes(nblk_pass))
    gidx = nc.dram_tensor("gidx", [gidx_len], dt.int16, kind="ExternalInput")
    dstl_in = nc.dram_tensor("dstl", [P * nblk], dt.float32, kind="ExternalInput")
    w_in = nc.dram_tensor("w", [P * nblk], dt.float32, kind="ExternalInput")
    iota_in = nc.dram_tensor("iota", [P * P], dt.float32, kind="ExternalInput")
    ws1 = nc.dram_tensor("W_self1", [F, F], dt.float32, kind="ExternalInput")
    wn1 = nc.dram_tensor("W_neigh1", [F, F], dt.float32, kind="ExternalInput")
    b1 = nc.dram_tensor("b1", [F], dt.float32, kind="ExternalInput")
    ws2 = nc.dram_tensor("W_self2", [F, F], dt.float32, kind="ExternalInput")
    wn2 = nc.dram_tensor("W_neigh2", [F, F], dt.float32, kind="ExternalInput")
    b2 = nc.dram_tensor("b2", [F], dt.float32, kind="ExternalInput")
    out_shard = nc.dram_tensor("out_shard", [OWN_PAD, F], dt.float32,
                               kind="ExternalOutput")

    h1_own = nc.dram_tensor("h1_own", [OWN_PAD, F], dt.float32)
    h1_rep = nc.dram_tensor("h1_rep", [N_PAD, F], dt.float32,
                            addr_space="Shared")

    pass_len = [min(CHUNK, N - p * CHUNK) for p in range(NPASS)]
    firsts, lasts = _schedule_flags(B)
    inst_sizes = _gather_instruction_sizes(nblk_pass)

    with tile.TileContext(nc) as tc:
        with (
            tc.tile_pool(name="const", bufs=1) as cpool,
            tc.tile_pool(name="gather", bufs=6) as gpool,
            tc.tile_pool(name="sel", bufs=6) as spool,
            tc.tile_pool(name="acc", bufs=1) as apool,
            tc.tile_pool(name="stage", bufs=3) as stpool,
            tc.tile_pool(name="psA", bufs=2, space="PSUM") as ppoolA,
            tc.tile_pool(name="psB", bufs=2, space="PSUM") as ppoolB,
        ):
            lib = nc.gpsimd.load_library(library_config.mlp)
            rows_regs = {}

            def rows_reg(v):
                if v not in rows_regs:
                    rows_regs[v] = nc.gpsimd.to_reg(v)
                return rows_regs[v]

            iota = cpool.tile([P, P], dt.float32)
            nc.sync.dma_start(out=iota[:],
                              in_=iota_in.ap().rearrange("(p f) -> p f", p=P))
            ident = cpool.tile([P, P], dt.float32)
            make_identity(nc, ident[:])
            ident_bf = cpool.tile([P, P], dt.bfloat16)
            nc.vector.tensor_copy(out=ident_bf[:], in_=ident[:])

            wtiles = {}
            for name, t in (("ws1", ws1), ("wn1", wn1), ("ws2", ws2),
                            ("wn2", wn2)):
                wt = cpool.tile([P, P], dt.float32, name=f"w_{name}", tag=f"w_{name}")
                nc.sync.dma_start(out=wt[:], in_=t[:, :])
                wtiles[name] = wt
            btiles = {}
            for name, t in (("b1", b1), ("b2", b2)):
                bt = cpool.tile([P, 1], dt.float32, name=f"b_{name}", tag=f"b_{name}")
                nc.sync.dma_start(out=bt[:], in_=t.ap()[:, None])
                btiles[name] = bt

            gidx_t = cpool.tile([P, gidx_len // P], dt.int16)
            nc.sync.dma_start(out=gidx_t[:],
                              in_=gidx.ap().rearrange("(p k) -> p k", p=P))
            dstl_t = cpool.tile([P, nblk], dt.float32)
            nc.sync.dma_start(out=dstl_t[:],
                              in_=dstl_in.ap().rearrange("(p b) -> p b", p=P))
            w_t = cpool.tile([P, nblk], dt.float32)
            nc.sync.dma_start(out=w_t[:],
                              in_=w_in.ap().rearrange("(p b) -> p b", p=P))

            aggT = apool.tile([P, NTILES * P], dt.float32)
            selfT = apool.tile([P, NTILES * P], dt.float32)

            def run_layer(table, self_table, wself, wneigh, bias,
                          dest, tdt, ddt, identt):
                nc.vector.memset(aggT[:], 0.0)

                live_psum = {}
                blk_cursor = 0      # global block index
                gcol = 0            # idx columns consumed in gidx_t
                pass_blk = 0        # block index within current pass
                cur_pass = 0
                ginst = 0           # gather instruction counter
                for pp, take in inst_sizes:
                    if pp != cur_pass:
                        cur_pass = pp
                        pass_blk = 0
                    rows = take * BLK
                    icols = rows // 16
                    gt = gpool.tile([P, (GBS // BLK) * P], tdt, tag="g")
                    g = nc.gpsimd.dma_gather(
                        gt[:, : take * P].rearrange("p (b f) -> p b f", f=P),
                        table[pp * CHUNK : pp * CHUNK + pass_len[pp], :],
                        gidx_t[:, gcol : gcol + icols],
                        rows,
                        rows_reg(rows),
                        F,
                        queue_num=ginst % 4,
                    )
                    gcol += icols
                    ginst += 1
                    add_dep_helper(g.ins, lib.ins, sync=False,
                                   reason="ucode lib before gather")

                    for k in range(take):
                        b = blk_cursor + k
                        t = int(blk_tile[b])
                        sel = spool.tile([P, P], tdt, tag="sel")
                        nc.vector.tensor_scalar(
                            sel[:], iota[:],
                            dstl_t[:, b : b + 1], w_t[:, b : b + 1],
                            mybir.AluOpType.is_equal, mybir.AluOpType.mult,
                        )
                        if firsts[pp][pass_blk + k]:
                            live_psum[t] = ppoolA.tile([P, P], dt.float32, name="ps",
                                                       tag="ps", space="PSUM")
                        ps = live_psum[t]
                        nc.tensor.matmul(
                            out=ps[:], lhsT=gt[:, k * P : (k + 1) * P],
                            rhs=sel[:],
                            start=bool(firsts[pp][pass_blk + k]),
                            stop=bool(lasts[pp][pass_blk + k]),
                        )
                        if lasts[pp][pass_blk + k]:
                            nc.vector.tensor_tensor(
                                out=aggT[:, t * P : (t + 1) * P],
                                in0=aggT[:, t * P : (t + 1) * P],
                                in1=ps[:], op=mybir.AluOpType.add,
                            )
                            del live_psum[t]
                    blk_cursor += take
                    pass_blk += take

                for t in range(NTILES):
                    xt = stpool.tile([P, P], tdt, tag="xt")
                    nc.sync.dma_start(out=xt[:],
                                      in_=self_table[t * P : (t + 1) * P, :])
                    pst = ppoolB.tile([P, P], tdt, tag="pst", space="PSUM")
                    nc.tensor.transpose(out=pst[:], in_=xt[:],
                                        identity=identt[:])
                    nc.vector.tensor_copy(out=selfT[:, t * P : (t + 1) * P],
                                          in_=pst[:])

                writes = []
                for g0 in range(0, NTILES, 4):
                    tn = min(4, NTILES - g0)
                    wdt = tn * P
                    psT = ppoolB.tile([P, 512], dt.float32, tag="psT",
                                      space="PSUM")
                    nc.tensor.matmul(out=psT[:, :wdt], lhsT=wneigh[:],
                                     rhs=aggT[:, g0 * P : g0 * P + wdt],
                                     start=True, stop=False)
                    nc.tensor.matmul(out=psT[:, :wdt], lhsT=wself[:],
                                     rhs=selfT[:, g0 * P : g0 * P + wdt],
                                     start=False, stop=True)
                    oT = stpool.tile([P, 512], dt.float32, tag="oT")
                    nc.scalar.activation(oT[:, :wdt], psT[:, :wdt],
                                         mybir.ActivationFunctionType.Relu,
                                         bias=bias[:, :1])
                    ost = stpool.tile([P, 512], ddt, tag="ost")
                    for j in range(tn):
                        psX = ppoolA.tile([P, P], dt.float32, tag="psX",
                                          space="PSUM")
                        nc.tensor.transpose(out=psX[:],
                                            in_=oT[:, j * P : (j + 1) * P],
                                            identity=ident[:])
                        nc.vector.tensor_copy(
                            out=ost[:, j * P : (j + 1) * P], in_=psX[:])
                    dd = nc.sync.dma_start(
                        out=dest[g0 * P : g0 * P + wdt, :]
                        .rearrange("(j p) f -> p j f", p=P),
                        in_=ost[:, :wdt].rearrange("p (j f) -> p j f", f=P),
                    )
                    writes.append(dd)
                return writes

            run_layer(x_rep, x_self, wtiles["ws1"], wtiles["wn1"],
                      btiles["b1"], h1_own, dt.float32, dt.float32, ident)

            if skip_collective:
                nc.sync.dma_start(out=h1_rep[0:OWN, :], in_=h1_own[0:OWN, :])
            else:
                nc.gpsimd.collective_compute(
                    "AllGather",
                    mybir.AluOpType.bypass,
                    replica_groups=[list(range(NCORES))],
                    ins=[h1_own[0:OWN, :]],
                    outs=[h1_rep[0:N, :]],
                )
            if N_PAD > N:
                zt = stpool.tile([P, F], dt.float32, tag="zt")
                nc.vector.memset(zt[:], 0.0)
                nc.sync.dma_start(out=h1_rep[N:N_PAD, :],
                                  in_=zt[: N_PAD - N, :])

            run_layer(h1_rep, h1_own, wtiles["ws2"], wtiles["wn2"],
                      btiles["b2"], out_shard, dt.float32, dt.float32, ident)

    _split_multi_waits(nc)
    from concourse.library_overlay import lower_extended_insts
    lower_extended_insts(nc)
    return nc


def _split_multi_waits(nc):
    pass_impl = True
    """Walrus codegen encodes at most one sync wait per instruction; split
    extras into standalone EventSemaphore instructions on the same in-order
    engine queue (semantically identical)."""
    import concourse.mybir as mybir

    n = 0
    for f in nc.m.functions:
        for b in f.blocks:
            insts = b.instructions
            new_list = []
            for inst in insts:
                si = inst.sync_info
                if si is not None and len(si.on_wait) > 1:
                    waits = list(si.on_wait)
                    for wt in waits[:-1]:
                        ev = mybir.InstEventSemaphore(
                            name=f"evsplit-{n}",
                            engine=inst.engine,
                            sync_info=mybir.SyncInfo(on_wait=[wt],
                                                     on_update=[]),
                            ins=[], outs=[],
                        )
                        new_list.append(ev)
                        try:
                            nc.inst_map[ev.name] = ev
                        except Exception:
                            pass
                        n += 1
                    inst.sync_info = mybir.SyncInfo(
                        on_wait=[waits[-1]], on_update=list(si.on_update)
                    )
                new_list.append(inst)
            insts[:] = new_list
    return n


# --------------------------------------------------------------------------
# entry point
# --------------------------------------------------------------------------

def prepare(x, edge_src, edge_dst, W_self1, W_neigh1, b1, W_self2, W_neigh2,
            b2):
    """Build the Bass program + per-core input maps (no execution)."""
    x = np.asarray(x, dtype=np.float32)
    plans, B, blk_tile, nblk_pass, nblk = _plan(edge_src, edge_dst)

    xpad = np.zeros((N_PAD, F), dtype=np.float32)
    xpad[:N] = x
    iota = np.broadcast_to(np.arange(P, dtype=np.float32), (P, P))

    in_maps = []
    for c in range(NCORES):
        pl = plans[c]
        xs = np.zeros((OWN_PAD, F), dtype=np.float32)
        xs[:OWN] = x[c * OWN : (c + 1) * OWN]
        in_maps.append({
            "x_rep": xpad,
            "x_self": xs,
            "gidx": _pack_gidx(pl["idx16"], nblk_pass),
            "dstl": pl["dstl"].reshape(nblk, P).T.copy().ravel(),
            "w": pl["w"].reshape(nblk, P).T.copy().ravel(),
            "iota": np.ascontiguousarray(iota).ravel(),
            "W_self1": np.asarray(W_self1, np.float32),
            "W_neigh1": np.asarray(W_neigh1, np.float32),
            "b1": np.asarray(b1, np.float32),
            "W_self2": np.asarray(W_self2, np.float32),
            "W_neigh2": np.asarray(W_neigh2, np.float32),
            "b2": np.asarray(b2, np.float32),
        })

    nc = _build(B, blk_tile, nblk_pass, nblk)
    return nc, in_maps


def assemble(results):
    return np.concatenate(
        [results[c]["out_shard"][:OWN] for c in range(NCORES)], axis=0
    ).astype(np.float32)


def kernel(x, edge_src, edge_dst, W_self1, W_neigh1, b1, W_self2, W_neigh2,
           b2, trace=False, _return_res=False):
    from concourse.bass_utils import run_bass_kernel_spmd

    nc, in_maps = prepare(x, edge_src, edge_dst, W_self1, W_neigh1, b1,
                          W_self2, W_neigh2, b2)
    res = run_bass_kernel_spmd(nc, in_maps, list(range(NCORES)), trace=trace)
    out = assemble(res.results)
    if _return_res:
        return out, res
    return out



# revision 10
# speedup vs baseline: 2.0121x; 2.0121x over previous
"""GraphSAGE-mean 2-layer GNN on 8 Trainium2 NeuronCores (Bass/Tile).

Sharding: nodes split into 8 contiguous ranges (rows c*12500..): core c
computes output rows for its range.  The full feature table is replicated per
core; layer-1 results are AllGather'd to rebuild the replicated table for
layer 2.

Aggregation: per core, edges (grouped by dst) are split into 4 passes by src
chunk of 32768 rows so src indices fit the int16 index format of the custom
dma_gather ucode (4096 rows per instruction).  Segment-sum runs on the tensor
engine: for each 128-edge block a selection matrix
  sel[e, m] = (dstl[e] == m) * invdeg[dst[e]]
is built in one fused DVE op from a constant iota tile, and
  psum[f, m] += msgs[e, f]^T @ sel[e, m]
accumulates weighted neighbor sums for one 128-node tile, feature-major.
The self path is contiguous loads + PE transpose; the transform computes
out^T = W_neigh^T @ aggT + W_self^T @ selfT with bias+relu fused into one
ScalarE activation, then PE-transposes back to node-major rows.

The SPMD program is shared by all 8 cores, so per-(pass, tile) block counts
are static = max over the 8 cores; shorter cores pad with zero-weight slots.
"""

import numpy as np

N = 100000
F = 128
NCORES = 8
OWN = N // NCORES            # 12500
P = 128
NTILES = (OWN + P - 1) // P  # 98
OWN_PAD = NTILES * P         # 12544
N_PAD = 100096               # table rows padded to a multiple of 128
CHUNK = 32768
NPASS = (N + CHUNK - 1) // CHUNK  # 4
GBS = 1024                   # gather rows per dma_gather instruction (SWDGE ring holds 1024 descs)
BLK = 128                    # edges per block


# --------------------------------------------------------------------------
# host-side planning
# --------------------------------------------------------------------------

def _plan(edge_src, edge_dst):
    src = np.asarray(edge_src).astype(np.int64).ravel()
    dst = np.asarray(edge_dst).astype(np.int64).ravel()
    deg = np.bincount(dst, minlength=N)
    invdeg = (1.0 / np.maximum(deg, 1)).astype(np.float32)

    per_core = []
    owner = dst // OWN
    for c in range(NCORES):
        m = owner == c
        s, d = src[m], dst[m]
        p = s // CHUNK
        order = np.lexsort((d, p))
        per_core.append((s[order], d[order], p[order]))

    cnt = np.zeros((NCORES, NPASS, NTILES), dtype=np.int64)
    for c in range(NCORES):
        s, d, p = per_core[c]
        t = (d - c * OWN) // P
        np.add.at(cnt, (c, p, t), 1)
    B = np.ceil(cnt.max(axis=0) / BLK).astype(np.int64)   # [NPASS, NTILES]

    nblk_pass = B.sum(axis=1).astype(np.int64)
    nblk = int(nblk_pass.sum())
    blk_tile = np.concatenate(
        [np.repeat(np.arange(NTILES), B[p]) for p in range(NPASS)]
    ).astype(np.int64)

    plans = []
    for c in range(NCORES):
        s, d, p = per_core[c]
        idx16 = np.zeros(nblk * BLK, dtype=np.int16)
        dstl = np.full(nblk * BLK, -1.0, dtype=np.float32)
        w = np.zeros(nblk * BLK, dtype=np.float32)
        blk0 = 0
        for pp in range(NPASS):
            m = p == pp
            sp, dp = s[m], d[m]
            tp = (dp - c * OWN) // P
            for t in range(NTILES):
                bcount = int(B[pp, t])
                if bcount == 0:
                    continue
                em = tp == t
                se, de = sp[em], dp[em]
                ne = se.shape[0]
                assert ne <= bcount * BLK
                base = blk0 * BLK
                idx16[base : base + ne] = (se - pp * CHUNK).astype(np.int16)
                dstl[base : base + ne] = (de - c * OWN - t * P).astype(np.float32)
                w[base : base + ne] = invdeg[de]
                blk0 += bcount
        assert blk0 == nblk
        plans.append({"idx16": idx16, "dstl": dstl, "w": w})

    return plans, B, blk_tile, nblk_pass, nblk


def _gather_instruction_sizes(nblk_pass):
    """Mirror of the device loop: list of (pass, blocks) per gather inst."""
    out = []
    for pp in range(NPASS):
        nb = int(nblk_pass[pp])
        done = 0
        while done < nb:
            take = min(GBS // BLK, nb - done)
            out.append((pp, take))
            done += take
    return out


def _pack_gidx(idx16, nblk_pass):
    """Pack int16 indices in the dma_gather SBUF layout (position j ->
    partition j%16, column j//16, replicated to 128 partitions) as one
    [128, total_cols] plane with per-instruction column segments, raveled
    partition-major.  Loaded to SBUF once and sliced per instruction."""
    total_cols = sum(take * BLK // 16
                     for _pp, take in _gather_instruction_sizes(nblk_pass))
    out = np.zeros((128, total_cols), dtype=np.int16)
    cursor = 0
    col = 0
    for _pp, take in _gather_instruction_sizes(nblk_pass):
        rows = take * BLK
        seg = idx16[cursor : cursor + rows]
        cursor += rows
        w16 = seg.reshape(rows // 16, 16).T          # [16, cols]
        out[:, col : col + rows // 16] = np.tile(w16, (8, 1))
        col += rows // 16
    return out.ravel()


def _schedule_flags(B):
    """start/stop flags per block within each pass (blocks are emitted
    pass-major, grouped by tile)."""
    firsts, lasts = [], []
    for pp in range(NPASS):
        tiles = [int(t) for t in np.repeat(np.arange(NTILES), B[pp])]
        f = [i == 0 or tiles[i] != tiles[i - 1] for i in range(len(tiles))]
        l = [i + 1 == len(tiles) or tiles[i + 1] != tiles[i]
             for i in range(len(tiles))]
        firsts.append(f)
        lasts.append(l)
    return firsts, lasts


# --------------------------------------------------------------------------
# device program
# --------------------------------------------------------------------------

def _build(B, blk_tile, nblk_pass, nblk, skip_collective=False):
    import concourse.bass as bass
    import concourse.mybir as mybir
    import concourse.tile as tile
    from concourse import library_config
    from concourse.masks import make_identity
    from concourse.tile_rust import add_dep_helper

    nc = bass.Bass("TRN2", target_bir_lowering=False, debug=False,
                   num_devices=NCORES, num_swdge_queues=4)
    dt = mybir.dt

    x_own = nc.dram_tensor("x_own", [OWN_PAD, F], dt.float32,
                           kind="ExternalInput")
    gidx_len = sum(128 * (take * BLK // 16)
                   for _pp, take in _gather_instruction_sizes(nblk_pass))
    gidx = nc.dram_tensor("gidx", [gidx_len], dt.int16, kind="ExternalInput")
    dstl_in = nc.dram_tensor("dstl", [P * nblk], dt.float32, kind="ExternalInput")
    w_in = nc.dram_tensor("w", [P * nblk], dt.float32, kind="ExternalInput")
    iota_in = nc.dram_tensor("iota", [P * P], dt.float32, kind="ExternalInput")
    ws1 = nc.dram_tensor("W_self1", [F, F], dt.float32, kind="ExternalInput")
    wn1 = nc.dram_tensor("W_neigh1", [F, F], dt.float32, kind="ExternalInput")
    b1 = nc.dram_tensor("b1", [F], dt.float32, kind="ExternalInput")
    ws2 = nc.dram_tensor("W_self2", [F, F], dt.float32, kind="ExternalInput")
    wn2 = nc.dram_tensor("W_neigh2", [F, F], dt.float32, kind="ExternalInput")
    b2 = nc.dram_tensor("b2", [F], dt.float32, kind="ExternalInput")
    out_shard = nc.dram_tensor("out_shard", [OWN_PAD, F], dt.float32,
                               kind="ExternalOutput")

    h1_own = nc.dram_tensor("h1_own", [OWN_PAD, F], dt.float32)
    h1_rep = nc.dram_tensor("h1_rep", [N_PAD, F], dt.float32,
                            addr_space="Shared")
    x_rep = nc.dram_tensor("x_rep", [N_PAD, F], dt.float32,
                           addr_space="Shared")
    # collectives cannot read ExternalInput tensors directly
    x_stage = nc.dram_tensor("x_stage", [OWN, F], dt.float32)

    pass_len = [min(CHUNK, N - p * CHUNK) for p in range(NPASS)]
    firsts, lasts = _schedule_flags(B)
    inst_sizes = _gather_instruction_sizes(nblk_pass)

    with tile.TileContext(nc) as tc:
        with (
            tc.tile_pool(name="const", bufs=1) as cpool,
            tc.tile_pool(name="gather", bufs=6) as gpool,
            tc.tile_pool(name="sel", bufs=6) as spool,
            tc.tile_pool(name="acc", bufs=1) as apool,
            tc.tile_pool(name="stage", bufs=3) as stpool,
            tc.tile_pool(name="psA", bufs=2, space="PSUM") as ppoolA,
            tc.tile_pool(name="psB", bufs=2, space="PSUM") as ppoolB,
        ):
            lib = nc.gpsimd.load_library(library_config.mlp)

            # Rebuild the replicated feature table on-device: binding a 51 MB
            # replicated input per core per call costs ~1 ms; the AllGather
            # of the 6.25 MB shards is far cheaper and overlaps with the
            # aux-table loads and the layer-1 self path below.
            if skip_collective:
                nc.sync.dma_start(out=x_rep[0:OWN, :], in_=x_own[0:OWN, :])
            else:
                nc.sync.dma_start(out=x_stage[:, :], in_=x_own[0:OWN, :])
                nc.gpsimd.collective_compute(
                    "AllGather",
                    mybir.AluOpType.bypass,
                    replica_groups=[list(range(NCORES))],
                    ins=[x_stage[:, :]],
                    outs=[x_rep[0:N, :]],
                )

            rows_regs = {}

            def rows_reg(v):
                if v not in rows_regs:
                    rows_regs[v] = nc.gpsimd.to_reg(v)
                return rows_regs[v]

            iota = cpool.tile([P, P], dt.float32)
            nc.sync.dma_start(out=iota[:],
                              in_=iota_in.ap().rearrange("(p f) -> p f", p=P))
            ident = cpool.tile([P, P], dt.float32)
            make_identity(nc, ident[:])
            ident_bf = cpool.tile([P, P], dt.bfloat16)
            nc.vector.tensor_copy(out=ident_bf[:], in_=ident[:])

            wtiles = {}
            for name, t in (("ws1", ws1), ("wn1", wn1), ("ws2", ws2),
                            ("wn2", wn2)):
                wt = cpool.tile([P, P], dt.float32, name=f"w_{name}", tag=f"w_{name}")
                nc.sync.dma_start(out=wt[:], in_=t[:, :])
                wtiles[name] = wt
            btiles = {}
            for name, t in (("b1", b1), ("b2", b2)):
                bt = cpool.tile([P, 1], dt.float32, name=f"b_{name}", tag=f"b_{name}")
                nc.sync.dma_start(out=bt[:], in_=t.ap()[:, None])
                btiles[name] = bt

            gidx_t = cpool.tile([P, gidx_len // P], dt.int16)
            nc.sync.dma_start(out=gidx_t[:],
                              in_=gidx.ap().rearrange("(p k) -> p k", p=P))
            dstl_t = cpool.tile([P, nblk], dt.float32)
            nc.sync.dma_start(out=dstl_t[:],
                              in_=dstl_in.ap().rearrange("(p b) -> p b", p=P))
            w_t = cpool.tile([P, nblk], dt.float32)
            nc.sync.dma_start(out=w_t[:],
                              in_=w_in.ap().rearrange("(p b) -> p b", p=P))

            aggT = apool.tile([P, NTILES * P], dt.float32)
            selfT = apool.tile([P, NTILES * P], dt.float32)

            def run_layer(table, self_table, wself, wneigh, bias,
                          dest, tdt, ddt, identt):
                nc.vector.memset(aggT[:], 0.0)

                live_psum = {}
                blk_cursor = 0      # global block index
                gcol = 0            # idx columns consumed in gidx_t
                pass_blk = 0        # block index within current pass
                cur_pass = 0
                ginst = 0           # gather instruction counter
                for pp, take in inst_sizes:
                    if pp != cur_pass:
                        cur_pass = pp
                        pass_blk = 0
                    rows = take * BLK
                    icols = rows // 16
                    gt = gpool.tile([P, (GBS // BLK) * P], tdt, tag="g")
                    g = nc.gpsimd.dma_gather(
                        gt[:, : take * P].rearrange("p (b f) -> p b f", f=P),
                        table[pp * CHUNK : pp * CHUNK + pass_len[pp], :],
                        gidx_t[:, gcol : gcol + icols],
                        rows,
                        rows_reg(rows),
                        F,
                        queue_num=ginst % 4,
                    )
                    gcol += icols
                    ginst += 1
                    add_dep_helper(g.ins, lib.ins, sync=False,
                                   reason="ucode lib before gather")

                    for k in range(take):
                        b = blk_cursor + k
                        t = int(blk_tile[b])
                        sel = spool.tile([P, P], tdt, tag="sel")
                        nc.vector.tensor_scalar(
                            sel[:], iota[:],
                            dstl_t[:, b : b + 1], w_t[:, b : b + 1],
                            mybir.AluOpType.is_equal, mybir.AluOpType.mult,
                        )
                        if firsts[pp][pass_blk + k]:
                            live_psum[t] = ppoolA.tile([P, P], dt.float32, name="ps",
                                                       tag="ps", space="PSUM")
                        ps = live_psum[t]
                        nc.tensor.matmul(
                            out=ps[:], lhsT=gt[:, k * P : (k + 1) * P],
                            rhs=sel[:],
                            start=bool(firsts[pp][pass_blk + k]),
                            stop=bool(lasts[pp][pass_blk + k]),
                        )
                        if lasts[pp][pass_blk + k]:
                            nc.vector.tensor_tensor(
                                out=aggT[:, t * P : (t + 1) * P],
                                in0=aggT[:, t * P : (t + 1) * P],
                                in1=ps[:], op=mybir.AluOpType.add,
                            )
                            del live_psum[t]
                    blk_cursor += take
                    pass_blk += take

                for t in range(NTILES):
                    xt = stpool.tile([P, P], tdt, tag="xt")
                    nc.sync.dma_start(out=xt[:],
                                      in_=self_table[t * P : (t + 1) * P, :])
                    pst = ppoolB.tile([P, P], tdt, tag="pst", space="PSUM")
                    nc.tensor.transpose(out=pst[:], in_=xt[:],
                                        identity=identt[:])
                    nc.vector.tensor_copy(out=selfT[:, t * P : (t + 1) * P],
                                          in_=pst[:])

                writes = []
                for g0 in range(0, NTILES, 4):
                    tn = min(4, NTILES - g0)
                    wdt = tn * P
                    psT = ppoolB.tile([P, 512], dt.float32, tag="psT",
                                      space="PSUM")
                    nc.tensor.matmul(out=psT[:, :wdt], lhsT=wneigh[:],
                                     rhs=aggT[:, g0 * P : g0 * P + wdt],
                                     start=True, stop=False)
                    nc.tensor.matmul(out=psT[:, :wdt], lhsT=wself[:],
                                     rhs=selfT[:, g0 * P : g0 * P + wdt],
                                     start=False, stop=True)
                    oT = stpool.tile([P, 512], dt.float32, tag="oT")
                    nc.scalar.activation(oT[:, :wdt], psT[:, :wdt],
                                         mybir.ActivationFunctionType.Relu,
                                         bias=bias[:, :1])
                    ost = stpool.tile([P, 512], ddt, tag="ost")
                    for j in range(tn):
                        psX = ppoolA.tile([P, P], dt.float32, tag="psX",
                                          space="PSUM")
                        nc.tensor.transpose(out=psX[:],
                                            in_=oT[:, j * P : (j + 1) * P],
                                            identity=ident[:])
                        nc.vector.tensor_copy(
                            out=ost[:, j * P : (j + 1) * P], in_=psX[:])
                    dd = nc.sync.dma_start(
                        out=dest[g0 * P : g0 * P + wdt, :]
                        .rearrange("(j p) f -> p j f", p=P),
                        in_=ost[:, :wdt].rearrange("p (j f) -> p j f", f=P),
                    )
                    writes.append(dd)
                return writes

            run_layer(x_rep, x_own, wtiles["ws1"], wtiles["wn1"],
                      btiles["b1"], h1_own, dt.float32, dt.float32, ident)

            if skip_collective:
                nc.sync.dma_start(out=h1_rep[0:OWN, :], in_=h1_own[0:OWN, :])
            else:
                nc.gpsimd.collective_compute(
                    "AllGather",
                    mybir.AluOpType.bypass,
                    replica_groups=[list(range(NCORES))],
                    ins=[h1_own[0:OWN, :]],
                    outs=[h1_rep[0:N, :]],
                )
            if N_PAD > N:
                zt = stpool.tile([P, F], dt.float32, tag="zt")
                nc.vector.memset(zt[:], 0.0)
                nc.sync.dma_start(out=h1_rep[N:N_PAD, :],
                                  in_=zt[: N_PAD - N, :])

            run_layer(h1_rep, h1_own, wtiles["ws2"], wtiles["wn2"],
                      btiles["b2"], out_shard, dt.float32, dt.float32, ident)

    _split_multi_waits(nc)
    from concourse.library_overlay import lower_extended_insts
    lower_extended_insts(nc)
    return nc


def _split_multi_waits(nc):
    pass_impl = True
    """Walrus codegen encodes at most one sync wait per instruction; split
    extras into standalone EventSemaphore instructions on the same in-order
    engine queue (semantically identical)."""
    import concourse.mybir as mybir

    n = 0
    for f in nc.m.functions:
        for b in f.blocks:
            insts = b.instructions
            new_list = []
            for inst in insts:
                si = inst.sync_info
                if si is not None and len(si.on_wait) > 1:
                    waits = list(si.on_wait)
                    for wt in waits[:-1]:
                        ev = mybir.InstEventSemaphore(
                            name=f"evsplit-{n}",
                            engine=inst.engine,
                            sync_info=mybir.SyncInfo(on_wait=[wt],
                                                     on_update=[]),
                            ins=[], outs=[],
                        )
                        new_list.append(ev)
                        try:
                            nc.inst_map[ev.name] = ev
                        except Exception:
                            pass
                        n += 1
                    inst.sync_info = mybir.SyncInfo(
                        on_wait=[waits[-1]], on_update=list(si.on_update)
                    )
                new_list.append(inst)
            insts[:] = new_list
    return n


# --------------------------------------------------------------------------
# entry point
# --------------------------------------------------------------------------

def prepare(x, edge_src, edge_dst, W_self1, W_neigh1, b1, W_self2, W_neigh2,
            b2):
    """Build the Bass program + per-core input maps (no execution)."""
    x = np.asarray(x, dtype=np.float32)
    plans, B, blk_tile, nblk_pass, nblk = _plan(edge_src, edge_dst)

    iota = np.broadcast_to(np.arange(P, dtype=np.float32), (P, P))

    in_maps = []
    for c in range(NCORES):
        pl = plans[c]
        xs = np.zeros((OWN_PAD, F), dtype=np.float32)
        xs[:OWN] = x[c * OWN : (c + 1) * OWN]
        in_maps.append({
            "x_own": xs,
            "gidx": _pack_gidx(pl["idx16"], nblk_pass),
            "dstl": pl["dstl"].reshape(nblk, P).T.copy().ravel(),
            "w": pl["w"].reshape(nblk, P).T.copy().ravel(),
            "iota": np.ascontiguousarray(iota).ravel(),
            "W_self1": np.asarray(W_self1, np.float32),
            "W_neigh1": np.asarray(W_neigh1, np.float32),
            "b1": np.asarray(b1, np.float32),
            "W_self2": np.asarray(W_self2, np.float32),
            "W_neigh2": np.asarray(W_neigh2, np.float32),
            "b2": np.asarray(b2, np.float32),
        })

    nc = _build(B, blk_tile, nblk_pass, nblk)
    return nc, in_maps


def assemble(results):
    return np.concatenate(
        [results[c]["out_shard"][:OWN] for c in range(NCORES)], axis=0
    ).astype(np.float32)


def kernel(x, edge_src, edge_dst, W_self1, W_neigh1, b1, W_self2, W_neigh2,
           b2, trace=False, _return_res=False):
    from concourse.bass_utils import run_bass_kernel_spmd

    nc, in_maps = prepare(x, edge_src, edge_dst, W_self1, W_neigh1, b1,
                          W_self2, W_neigh2, b2)
    res = run_bass_kernel_spmd(nc, in_maps, list(range(NCORES)), trace=trace)
    out = assemble(res.results)
    if _return_res:
        return out, res
    return out



# revision 17
# speedup vs baseline: 2.1855x; 1.0862x over previous
"""GraphSAGE-mean 2-layer GNN on 8 Trainium2 NeuronCores (Bass/Tile).

Sharding: nodes split into 8 contiguous ranges (rows c*12500..): core c
computes output rows for its range.  The full feature table is replicated per
core; layer-1 results are AllGather'd to rebuild the replicated table for
layer 2.

Aggregation: per core, edges (grouped by dst) are split into 4 passes by src
chunk of 32768 rows so src indices fit the int16 index format of the custom
dma_gather ucode (4096 rows per instruction).  Segment-sum runs on the tensor
engine: for each 128-edge block a selection matrix
  sel[e, m] = (dstl[e] == m) * invdeg[dst[e]]
is built in one fused DVE op from a constant iota tile, and
  psum[f, m] += msgs[e, f]^T @ sel[e, m]
accumulates weighted neighbor sums for one 128-node tile, feature-major.
The self path is contiguous loads + PE transpose; the transform computes
out^T = W_neigh^T @ aggT + W_self^T @ selfT with bias+relu fused into one
ScalarE activation, then PE-transposes back to node-major rows.

The SPMD program is shared by all 8 cores, so per-(pass, tile) block counts
are static = max over the 8 cores; shorter cores pad with zero-weight slots.
"""

import numpy as np

N = 100000
F = 128
NCORES = 8
OWN = N // NCORES            # 12500
P = 128
NTILES = (OWN + P - 1) // P  # 98
OWN_PAD = NTILES * P         # 12544
N_PAD = 100096               # table rows padded to a multiple of 128
CHUNK = 32768
NPASS = (N + CHUNK - 1) // CHUNK  # 4
GBS = 1024                   # gather rows per dma_gather instruction (SWDGE ring holds 1024 descs)
BLK = 128                    # edges per block


# --------------------------------------------------------------------------
# host-side planning
# --------------------------------------------------------------------------

def _plan(edge_src, edge_dst):
    src = np.asarray(edge_src).astype(np.int64).ravel()
    dst = np.asarray(edge_dst).astype(np.int64).ravel()
    deg = np.bincount(dst, minlength=N)
    invdeg = (1.0 / np.maximum(deg, 1)).astype(np.float32)

    per_core = []
    owner = dst // OWN
    for c in range(NCORES):
        m = owner == c
        s, d = src[m], dst[m]
        p = s // CHUNK
        order = np.lexsort((d, p))
        per_core.append((s[order], d[order], p[order]))

    cnt = np.zeros((NCORES, NPASS, NTILES), dtype=np.int64)
    for c in range(NCORES):
        s, d, p = per_core[c]
        t = (d - c * OWN) // P
        np.add.at(cnt, (c, p, t), 1)
    B = np.ceil(cnt.max(axis=0) / BLK).astype(np.int64)   # [NPASS, NTILES]

    nblk_pass = B.sum(axis=1).astype(np.int64)
    nblk = int(nblk_pass.sum())
    blk_tile = np.concatenate(
        [np.repeat(np.arange(NTILES), B[p]) for p in range(NPASS)]
    ).astype(np.int64)

    plans = []
    for c in range(NCORES):
        s, d, p = per_core[c]
        idx16 = np.zeros(nblk * BLK, dtype=np.int16)
        dstl = np.full(nblk * BLK, -1.0, dtype=np.float32)
        w = np.zeros(nblk * BLK, dtype=np.float32)
        blk0 = 0
        for pp in range(NPASS):
            m = p == pp
            sp, dp = s[m], d[m]
            tp = (dp - c * OWN) // P
            for t in range(NTILES):
                bcount = int(B[pp, t])
                if bcount == 0:
                    continue
                em = tp == t
                se, de = sp[em], dp[em]
                ne = se.shape[0]
                assert ne <= bcount * BLK
                base = blk0 * BLK
                idx16[base : base + ne] = (se - pp * CHUNK).astype(np.int16)
                dstl[base : base + ne] = (de - c * OWN - t * P).astype(np.float32)
                w[base : base + ne] = invdeg[de]
                blk0 += bcount
        assert blk0 == nblk
        plans.append({"idx16": idx16, "dstl": dstl, "w": w})

    return plans, B, blk_tile, nblk_pass, nblk


def _gather_instruction_sizes(nblk_pass):
    """Mirror of the device loop: list of (pass, blocks) per gather inst."""
    out = []
    for pp in range(NPASS):
        nb = int(nblk_pass[pp])
        done = 0
        while done < nb:
            take = min(GBS // BLK, nb - done)
            out.append((pp, take))
            done += take
    return out


def _pack_gidx(idx16, nblk_pass):
    """Pack int16 indices in the dma_gather SBUF layout (position j ->
    partition j%16, column j//16, replicated to 128 partitions) as one
    [128, total_cols] plane with per-instruction column segments, raveled
    partition-major.  Loaded to SBUF once and sliced per instruction."""
    total_cols = sum(take * BLK // 16
                     for _pp, take in _gather_instruction_sizes(nblk_pass))
    out = np.zeros((128, total_cols), dtype=np.int16)
    cursor = 0
    col = 0
    for _pp, take in _gather_instruction_sizes(nblk_pass):
        rows = take * BLK
        seg = idx16[cursor : cursor + rows]
        cursor += rows
        w16 = seg.reshape(rows // 16, 16).T          # [16, cols]
        out[:, col : col + rows // 16] = np.tile(w16, (8, 1))
        col += rows // 16
    return out.ravel()


def _schedule_flags(B):
    """start/stop flags per block within each pass (blocks are emitted
    pass-major, grouped by tile)."""
    firsts, lasts = [], []
    for pp in range(NPASS):
        tiles = [int(t) for t in np.repeat(np.arange(NTILES), B[pp])]
        f = [i == 0 or tiles[i] != tiles[i - 1] for i in range(len(tiles))]
        l = [i + 1 == len(tiles) or tiles[i + 1] != tiles[i]
             for i in range(len(tiles))]
        firsts.append(f)
        lasts.append(l)
    return firsts, lasts


# --------------------------------------------------------------------------
# device program
# --------------------------------------------------------------------------

def _build(B, blk_tile, nblk_pass, nblk, skip_collective=False):
    import concourse.bass as bass
    import concourse.mybir as mybir
    import concourse.tile as tile
    from concourse import library_config
    from concourse.masks import make_identity
    from concourse.tile_rust import add_dep_helper

    nc = bass.Bass("TRN2", target_bir_lowering=False, debug=False,
                   num_devices=NCORES, num_swdge_queues=4)
    dt = mybir.dt

    x_own = nc.dram_tensor("x_own", [OWN_PAD, F], dt.bfloat16,
                           kind="ExternalInput")
    gidx_len = sum(128 * (take * BLK // 16)
                   for _pp, take in _gather_instruction_sizes(nblk_pass))
    gidx = nc.dram_tensor("gidx", [gidx_len], dt.int16, kind="ExternalInput")
    dstl_in = nc.dram_tensor("dstl", [P * nblk], dt.float32, kind="ExternalInput")
    w_in = nc.dram_tensor("w", [P * nblk], dt.float32, kind="ExternalInput")
    iota_in = nc.dram_tensor("iota", [P * P], dt.float32, kind="ExternalInput")
    ws1 = nc.dram_tensor("W_self1", [F, F], dt.float32, kind="ExternalInput")
    wn1 = nc.dram_tensor("W_neigh1", [F, F], dt.float32, kind="ExternalInput")
    b1 = nc.dram_tensor("b1", [F], dt.float32, kind="ExternalInput")
    ws2 = nc.dram_tensor("W_self2", [F, F], dt.float32, kind="ExternalInput")
    wn2 = nc.dram_tensor("W_neigh2", [F, F], dt.float32, kind="ExternalInput")
    b2 = nc.dram_tensor("b2", [F], dt.float32, kind="ExternalInput")
    out_shard = nc.dram_tensor("out_shard", [OWN_PAD, F], dt.float32,
                               kind="ExternalOutput")

    h1_own = nc.dram_tensor("h1_own", [OWN_PAD, F], dt.bfloat16)
    h1_rep = nc.dram_tensor("h1_rep", [N_PAD, F], dt.bfloat16,
                            addr_space="Shared")
    x_rep = nc.dram_tensor("x_rep", [N_PAD, F], dt.bfloat16,
                           addr_space="Shared")
    # collectives cannot read ExternalInput tensors directly
    x_stage = nc.dram_tensor("x_stage", [OWN, F], dt.bfloat16)

    pass_len = [min(CHUNK, N - p * CHUNK) for p in range(NPASS)]
    firsts, lasts = _schedule_flags(B)
    inst_sizes = _gather_instruction_sizes(nblk_pass)

    with tile.TileContext(nc) as tc:
        with (
            tc.tile_pool(name="const", bufs=1) as cpool,
            tc.tile_pool(name="gather", bufs=6) as gpool,
            tc.tile_pool(name="sel", bufs=6) as spool,
            tc.tile_pool(name="acc", bufs=1) as apool,
            tc.tile_pool(name="stage", bufs=3) as stpool,
            tc.tile_pool(name="psA", bufs=2, space="PSUM") as ppoolA,
            tc.tile_pool(name="psB", bufs=2, space="PSUM") as ppoolB,
        ):
            lib = nc.gpsimd.load_library(library_config.mlp)

            # Rebuild the replicated feature table on-device: binding a 51 MB
            # replicated input per core per call costs ~1 ms; the AllGather
            # of the 6.25 MB shards is far cheaper and overlaps with the
            # aux-table loads and the layer-1 self path below.
            if skip_collective:
                nc.sync.dma_start(out=x_rep[0:OWN, :], in_=x_own[0:OWN, :])
            else:
                nc.sync.dma_start(out=x_stage[:, :], in_=x_own[0:OWN, :])
                nc.gpsimd.collective_compute(
                    "AllGather",
                    mybir.AluOpType.bypass,
                    replica_groups=[list(range(NCORES))],
                    ins=[x_stage[:, :]],
                    outs=[x_rep[0:N, :]],
                )

            rows_regs = {}

            def rows_reg(v):
                if v not in rows_regs:
                    rows_regs[v] = nc.gpsimd.to_reg(v)
                return rows_regs[v]

            iota = cpool.tile([P, P], dt.float32)
            nc.sync.dma_start(out=iota[:],
                              in_=iota_in.ap().rearrange("(p f) -> p f", p=P))
            ident = cpool.tile([P, P], dt.float32)
            make_identity(nc, ident[:])
            ident_bf = cpool.tile([P, P], dt.bfloat16)
            nc.vector.tensor_copy(out=ident_bf[:], in_=ident[:])

            wtiles = {}
            for name, t in (("ws1", ws1), ("wn1", wn1), ("ws2", ws2),
                            ("wn2", wn2)):
                wt = cpool.tile([P, P], dt.float32, name=f"w_{name}", tag=f"w_{name}")
                nc.sync.dma_start(out=wt[:], in_=t[:, :])
                wtiles[name] = wt
            btiles = {}
            for name, t in (("b1", b1), ("b2", b2)):
                bt = cpool.tile([P, 1], dt.float32, name=f"b_{name}", tag=f"b_{name}")
                nc.sync.dma_start(out=bt[:], in_=t.ap()[:, None])
                btiles[name] = bt

            gidx_t = cpool.tile([P, gidx_len // P], dt.int16)
            nc.sync.dma_start(out=gidx_t[:],
                              in_=gidx.ap().rearrange("(p k) -> p k", p=P))
            dstl_t = cpool.tile([P, nblk], dt.float32)
            nc.sync.dma_start(out=dstl_t[:],
                              in_=dstl_in.ap().rearrange("(p b) -> p b", p=P))
            w_t = cpool.tile([P, nblk], dt.float32)
            nc.sync.dma_start(out=w_t[:],
                              in_=w_in.ap().rearrange("(p b) -> p b", p=P))

            aggT = apool.tile([P, NTILES * P], dt.float32)
            selfT = apool.tile([P, NTILES * P], dt.float32)

            def run_layer(table, self_table, wself, wneigh, bias,
                          dest, tdt, ddt, identt):
                nc.vector.memset(aggT[:], 0.0)

                live_psum = {}
                blk_cursor = 0      # global block index
                gcol = 0            # idx columns consumed in gidx_t
                pass_blk = 0        # block index within current pass
                cur_pass = 0
                ginst = 0           # gather instruction counter
                for pp, take in inst_sizes:
                    if pp != cur_pass:
                        cur_pass = pp
                        pass_blk = 0
                    rows = take * BLK
                    icols = rows // 16
                    gt = gpool.tile([P, (GBS // BLK) * P], tdt, tag="g")
                    g = nc.gpsimd.dma_gather(
                        gt[:, : take * P].rearrange("p (b f) -> p b f", f=P),
                        table[pp * CHUNK : pp * CHUNK + pass_len[pp], :],
                        gidx_t[:, gcol : gcol + icols],
                        rows,
                        rows_reg(rows),
                        F,
                        queue_num=ginst % 4,
                    )
                    gcol += icols
                    ginst += 1
                    add_dep_helper(g.ins, lib.ins, sync=False,
                                   reason="ucode lib before gather")

                    for k in range(take):
                        b = blk_cursor + k
                        t = int(blk_tile[b])
                        sel = spool.tile([P, P], tdt, tag="sel")
                        nc.vector.tensor_scalar(
                            sel[:], iota[:],
                            dstl_t[:, b : b + 1], w_t[:, b : b + 1],
                            mybir.AluOpType.is_equal, mybir.AluOpType.mult,
                        )
                        if firsts[pp][pass_blk + k]:
                            live_psum[t] = ppoolA.tile([P, P], dt.float32, name="ps",
                                                       tag="ps", space="PSUM")
                        ps = live_psum[t]
                        nc.tensor.matmul(
                            out=ps[:], lhsT=gt[:, k * P : (k + 1) * P],
                            rhs=sel[:],
                            start=bool(firsts[pp][pass_blk + k]),
                            stop=bool(lasts[pp][pass_blk + k]),
                        )
                        if lasts[pp][pass_blk + k]:
                            nc.vector.tensor_tensor(
                                out=aggT[:, t * P : (t + 1) * P],
                                in0=aggT[:, t * P : (t + 1) * P],
                                in1=ps[:], op=mybir.AluOpType.add,
                            )
                            del live_psum[t]
                    blk_cursor += take
                    pass_blk += take

                for t in range(NTILES):
                    xt = stpool.tile([P, P], tdt, tag="xt")
                    nc.sync.dma_start(out=xt[:],
                                      in_=self_table[t * P : (t + 1) * P, :])
                    pst = ppoolB.tile([P, P], tdt, tag="pst", space="PSUM")
                    nc.tensor.transpose(out=pst[:], in_=xt[:],
                                        identity=identt[:])
                    nc.vector.tensor_copy(out=selfT[:, t * P : (t + 1) * P],
                                          in_=pst[:])

                writes = []
                for g0 in range(0, NTILES, 4):
                    tn = min(4, NTILES - g0)
                    wdt = tn * P
                    psT = ppoolB.tile([P, 512], dt.float32, tag="psT",
                                      space="PSUM")
                    nc.tensor.matmul(out=psT[:, :wdt], lhsT=wneigh[:],
                                     rhs=aggT[:, g0 * P : g0 * P + wdt],
                                     start=True, stop=False)
                    nc.tensor.matmul(out=psT[:, :wdt], lhsT=wself[:],
                                     rhs=selfT[:, g0 * P : g0 * P + wdt],
                                     start=False, stop=True)
                    oT = stpool.tile([P, 512], dt.float32, tag="oT")
                    nc.scalar.activation(oT[:, :wdt], psT[:, :wdt],
                                         mybir.ActivationFunctionType.Relu,
                                         bias=bias[:, :1])
                    ost = stpool.tile([P, 512], ddt, tag="ost")
                    for j in range(tn):
                        psX = ppoolA.tile([P, P], dt.float32, tag="psX",
                                          space="PSUM")
                        nc.tensor.transpose(out=psX[:],
                                            in_=oT[:, j * P : (j + 1) * P],
                                            identity=ident[:])
                        nc.vector.tensor_copy(
                            out=ost[:, j * P : (j + 1) * P], in_=psX[:])
                    dd = nc.sync.dma_start(
                        out=dest[g0 * P : g0 * P + wdt, :]
                        .rearrange("(j p) f -> p j f", p=P),
                        in_=ost[:, :wdt].rearrange("p (j f) -> p j f", f=P),
                    )
                    writes.append(dd)
                return writes

            run_layer(x_rep, x_own, wtiles["ws1"], wtiles["wn1"],
                      btiles["b1"], h1_own, dt.bfloat16, dt.bfloat16,
                      ident_bf)

            if skip_collective:
                nc.sync.dma_start(out=h1_rep[0:OWN, :], in_=h1_own[0:OWN, :])
            else:
                nc.gpsimd.collective_compute(
                    "AllGather",
                    mybir.AluOpType.bypass,
                    replica_groups=[list(range(NCORES))],
                    ins=[h1_own[0:OWN, :]],
                    outs=[h1_rep[0:N, :]],
                )
            if N_PAD > N:
                zt = stpool.tile([P, F], dt.bfloat16, tag="zt")
                nc.vector.memset(zt[:], 0.0)
                nc.sync.dma_start(out=h1_rep[N:N_PAD, :],
                                  in_=zt[: N_PAD - N, :])

            run_layer(h1_rep, h1_own, wtiles["ws2"], wtiles["wn2"],
                      btiles["b2"], out_shard, dt.bfloat16, dt.float32,
                      ident_bf)

    _split_multi_waits(nc)
    from concourse.library_overlay import lower_extended_insts
    lower_extended_insts(nc)
    return nc


def _split_multi_waits(nc):
    pass_impl = True
    """Walrus codegen encodes at most one sync wait per instruction; split
    extras into standalone EventSemaphore instructions on the same in-order
    engine queue (semantically identical)."""
    import concourse.mybir as mybir

    n = 0
    for f in nc.m.functions:
        for b in f.blocks:
            insts = b.instructions
            new_list = []
            for inst in insts:
                si = inst.sync_info
                if si is not None and len(si.on_wait) > 1:
                    waits = list(si.on_wait)
                    for wt in waits[:-1]:
                        ev = mybir.InstEventSemaphore(
                            name=f"evsplit-{n}",
                            engine=inst.engine,
                            sync_info=mybir.SyncInfo(on_wait=[wt],
                                                     on_update=[]),
                            ins=[], outs=[],
                        )
                        new_list.append(ev)
                        try:
                            nc.inst_map[ev.name] = ev
                        except Exception:
                            pass
                        n += 1
                    inst.sync_info = mybir.SyncInfo(
                        on_wait=[waits[-1]], on_update=list(si.on_update)
                    )
                new_list.append(inst)
            insts[:] = new_list
    return n


# --------------------------------------------------------------------------
# entry point
# --------------------------------------------------------------------------

def prepare(x, edge_src, edge_dst, W_self1, W_neigh1, b1, W_self2, W_neigh2,
            b2):
    """Build the Bass program + per-core input maps (no execution)."""
    x = np.asarray(x, dtype=np.float32)
    plans, B, blk_tile, nblk_pass, nblk = _plan(edge_src, edge_dst)

    import ml_dtypes

    bf16 = ml_dtypes.bfloat16
    iota = np.broadcast_to(np.arange(P, dtype=np.float32), (P, P))

    in_maps = []
    for c in range(NCORES):
        pl = plans[c]
        xs = np.zeros((OWN_PAD, F), dtype=bf16)
        xs[:OWN] = x[c * OWN : (c + 1) * OWN].astype(bf16)
        in_maps.append({
            "x_own": xs,
            "gidx": _pack_gidx(pl["idx16"], nblk_pass),
            "dstl": pl["dstl"].reshape(nblk, P).T.copy().ravel(),
            "w": pl["w"].reshape(nblk, P).T.copy().ravel(),
            "iota": np.ascontiguousarray(iota).ravel(),
            "W_self1": np.asarray(W_self1, np.float32),
            "W_neigh1": np.asarray(W_neigh1, np.float32),
            "b1": np.asarray(b1, np.float32),
            "W_self2": np.asarray(W_self2, np.float32),
            "W_neigh2": np.asarray(W_neigh2, np.float32),
            "b2": np.asarray(b2, np.float32),
        })

    nc = _build(B, blk_tile, nblk_pass, nblk)
    return nc, in_maps


def assemble(results):
    return np.concatenate(
        [results[c]["out_shard"][:OWN] for c in range(NCORES)], axis=0
    ).astype(np.float32)


def kernel(x, edge_src, edge_dst, W_self1, W_neigh1, b1, W_self2, W_neigh2,
           b2, trace=False, _return_res=False):
    from concourse.bass_utils import run_bass_kernel_spmd

    nc, in_maps = prepare(x, edge_src, edge_dst, W_self1, W_neigh1, b1,
                          W_self2, W_neigh2, b2)
    res = run_bass_kernel_spmd(nc, in_maps, list(range(NCORES)), trace=trace)
    out = assemble(res.results)
    if _return_res:
        return out, res
    return out



# revision 24
# speedup vs baseline: 2.9009x; 1.3274x over previous
"""GraphSAGE-mean 2-layer GNN on 8 Trainium2 NeuronCores (Bass/Tile).

Sharding: nodes split into 8 contiguous ranges (rows c*12500..): core c
computes output rows for its range.  The full feature table is replicated per
core; layer-1 results are AllGather'd to rebuild the replicated table for
layer 2.

Aggregation: per core, edges (grouped by dst) are split into 4 passes by src
chunk of 32768 rows so src indices fit the int16 index format of the custom
dma_gather ucode (4096 rows per instruction).  Segment-sum runs on the tensor
engine: for each 128-edge block a selection matrix
  sel[e, m] = (dstl[e] == m) * invdeg[dst[e]]
is built in one fused DVE op from a constant iota tile, and
  psum[f, m] += msgs[e, f]^T @ sel[e, m]
accumulates weighted neighbor sums for one 128-node tile, feature-major.
The self path is contiguous loads + PE transpose; the transform computes
out^T = W_neigh^T @ aggT + W_self^T @ selfT with bias+relu fused into one
ScalarE activation, then PE-transposes back to node-major rows.

The SPMD program is shared by all 8 cores, so per-(pass, tile) block counts
are static = max over the 8 cores; shorter cores pad with zero-weight slots.
"""

import numpy as np

N = 100000
F = 128
NCORES = 8
OWN = N // NCORES            # 12500
P = 128
NTILES = (OWN + P - 1) // P  # 98
OWN_PAD = NTILES * P         # 12544
N_PAD = 100096               # table rows padded to a multiple of 128
CHUNK = 32768
NPASS = (N + CHUNK - 1) // CHUNK  # 4
GBS = 128                    # gather rows per dma_gather instruction; small
                             # gathers complete independently, so consumers
                             # start sooner and the 4 SWDGE queues stay busy
BLK = 128                    # edges per block


# --------------------------------------------------------------------------
# host-side planning
# --------------------------------------------------------------------------

def _plan(edge_src, edge_dst):
    src = np.asarray(edge_src).astype(np.int64).ravel()
    dst = np.asarray(edge_dst).astype(np.int64).ravel()
    deg = np.bincount(dst, minlength=N)
    invdeg = (1.0 / np.maximum(deg, 1)).astype(np.float32)

    per_core = []
    owner = dst // OWN
    for c in range(NCORES):
        m = owner == c
        s, d = src[m], dst[m]
        p = s // CHUNK
        order = np.lexsort((d, p))
        per_core.append((s[order], d[order], p[order]))

    cnt = np.zeros((NCORES, NPASS, NTILES), dtype=np.int64)
    for c in range(NCORES):
        s, d, p = per_core[c]
        t = (d - c * OWN) // P
        np.add.at(cnt, (c, p, t), 1)
    B = np.ceil(cnt.max(axis=0) / BLK).astype(np.int64)   # [NPASS, NTILES]

    nblk_pass = B.sum(axis=1).astype(np.int64)
    nblk = int(nblk_pass.sum())
    blk_tile = np.concatenate(
        [np.repeat(np.arange(NTILES), B[p]) for p in range(NPASS)]
    ).astype(np.int64)

    plans = []
    for c in range(NCORES):
        s, d, p = per_core[c]
        idx16 = np.zeros(nblk * BLK, dtype=np.int16)
        dstl = np.full(nblk * BLK, -1.0, dtype=np.float32)
        w = np.zeros(nblk * BLK, dtype=np.float32)
        blk0 = 0
        for pp in range(NPASS):
            m = p == pp
            sp, dp = s[m], d[m]
            tp = (dp - c * OWN) // P
            for t in range(NTILES):
                bcount = int(B[pp, t])
                if bcount == 0:
                    continue
                em = tp == t
                se, de = sp[em], dp[em]
                ne = se.shape[0]
                assert ne <= bcount * BLK
                base = blk0 * BLK
                idx16[base : base + ne] = (se - pp * CHUNK).astype(np.int16)
                dstl[base : base + ne] = (de - c * OWN - t * P).astype(np.float32)
                w[base : base + ne] = invdeg[de]
                blk0 += bcount
        assert blk0 == nblk
        plans.append({"idx16": idx16, "dstl": dstl, "w": w})

    return plans, B, blk_tile, nblk_pass, nblk


def _gather_instruction_sizes(nblk_pass):
    """Mirror of the device loop: list of (pass, blocks) per gather inst."""
    out = []
    for pp in range(NPASS):
        nb = int(nblk_pass[pp])
        done = 0
        while done < nb:
            take = min(GBS // BLK, nb - done)
            out.append((pp, take))
            done += take
    return out


def _pack_gidx(idx16, nblk_pass):
    """Pack int16 indices in the dma_gather SBUF layout (position j ->
    partition j%16, column j//16, replicated to 128 partitions) as one
    [128, total_cols] plane with per-instruction column segments, raveled
    partition-major.  Loaded to SBUF once and sliced per instruction."""
    total_cols = sum(take * BLK // 16
                     for _pp, take in _gather_instruction_sizes(nblk_pass))
    out = np.zeros((128, total_cols), dtype=np.int16)
    cursor = 0
    col = 0
    for _pp, take in _gather_instruction_sizes(nblk_pass):
        rows = take * BLK
        seg = idx16[cursor : cursor + rows]
        cursor += rows
        w16 = seg.reshape(rows // 16, 16).T          # [16, cols]
        out[:, col : col + rows // 16] = np.tile(w16, (8, 1))
        col += rows // 16
    return out.ravel()


def _schedule_flags(B):
    """start/stop flags per block within each pass (blocks are emitted
    pass-major, grouped by tile)."""
    firsts, lasts = [], []
    for pp in range(NPASS):
        tiles = [int(t) for t in np.repeat(np.arange(NTILES), B[pp])]
        f = [i == 0 or tiles[i] != tiles[i - 1] for i in range(len(tiles))]
        l = [i + 1 == len(tiles) or tiles[i + 1] != tiles[i]
             for i in range(len(tiles))]
        firsts.append(f)
        lasts.append(l)
    return firsts, lasts


# --------------------------------------------------------------------------
# device program
# --------------------------------------------------------------------------

def _build(B, blk_tile, nblk_pass, nblk, skip_collective=False,
           no_gather=False, no_agg=False, bind_only=False):
    import concourse.bass as bass
    import concourse.mybir as mybir
    import concourse.tile as tile
    from concourse import library_config
    from concourse.masks import make_identity
    from concourse.tile_rust import add_dep_helper

    nc = bass.Bass("TRN2", target_bir_lowering=False, debug=False,
                   num_devices=NCORES, num_swdge_queues=4)
    dt = mybir.dt

    x_own = nc.dram_tensor("x_own", [OWN_PAD, F], dt.bfloat16,
                           kind="ExternalInput")
    gidx_len = sum(128 * (take * BLK // 16)
                   for _pp, take in _gather_instruction_sizes(nblk_pass))
    gidx = nc.dram_tensor("gidx", [gidx_len], dt.int16, kind="ExternalInput")
    dstl_in = nc.dram_tensor("dstl", [P * nblk], dt.float32, kind="ExternalInput")
    w_in = nc.dram_tensor("w", [P * nblk], dt.float32, kind="ExternalInput")
    iota_in = nc.dram_tensor("iota", [P * P], dt.float32, kind="ExternalInput")
    ws1 = nc.dram_tensor("W_self1", [F, F], dt.float32, kind="ExternalInput")
    wn1 = nc.dram_tensor("W_neigh1", [F, F], dt.float32, kind="ExternalInput")
    b1 = nc.dram_tensor("b1", [F], dt.float32, kind="ExternalInput")
    ws2 = nc.dram_tensor("W_self2", [F, F], dt.float32, kind="ExternalInput")
    wn2 = nc.dram_tensor("W_neigh2", [F, F], dt.float32, kind="ExternalInput")
    b2 = nc.dram_tensor("b2", [F], dt.float32, kind="ExternalInput")
    out_shard = nc.dram_tensor("out_shard", [OWN_PAD, F], dt.float32,
                               kind="ExternalOutput")

    h1_own = nc.dram_tensor("h1_own", [OWN_PAD, F], dt.bfloat16)
    h1_rep = nc.dram_tensor("h1_rep", [N_PAD, F], dt.bfloat16,
                            addr_space="Shared")
    x_rep = nc.dram_tensor("x_rep", [N_PAD, F], dt.bfloat16,
                           addr_space="Shared")
    # collectives cannot read ExternalInput tensors directly
    x_stage = nc.dram_tensor("x_stage", [OWN, F], dt.bfloat16)

    pass_len = [min(CHUNK, N - p * CHUNK) for p in range(NPASS)]
    firsts, lasts = _schedule_flags(B)
    inst_sizes = _gather_instruction_sizes(nblk_pass)

    if bind_only:   # timing diagnostic: inputs bound, near-empty body
        with tile.TileContext(nc) as tc:
            with tc.tile_pool(name="p", bufs=1) as pool:
                t = pool.tile([P, F], dt.bfloat16)
                nc.sync.dma_start(out=t[:], in_=x_own[0:P, :])
                to = pool.tile([P, F], dt.float32)
                nc.vector.tensor_copy(out=to[:], in_=t[:])
                nc.sync.dma_start(out=out_shard[0:P, :], in_=to[:])
        _split_multi_waits(nc)
        from concourse.library_overlay import lower_extended_insts
        lower_extended_insts(nc)
        return nc

    with tile.TileContext(nc) as tc:
        with (
            tc.tile_pool(name="const", bufs=1) as cpool,
            tc.tile_pool(name="gather", bufs=10) as gpool,
            tc.tile_pool(name="sel", bufs=10) as spool,
            tc.tile_pool(name="acc", bufs=1) as apool,
            tc.tile_pool(name="stage", bufs=3) as stpool,
            tc.tile_pool(name="psA", bufs=2, space="PSUM") as ppoolA,
            tc.tile_pool(name="psB", bufs=2, space="PSUM") as ppoolB,
        ):
            lib = nc.gpsimd.load_library(library_config.mlp)

            # Rebuild the replicated feature table on-device: binding a 51 MB
            # replicated input per core per call costs ~1 ms; the AllGather
            # of the 6.25 MB shards is far cheaper and overlaps with the
            # aux-table loads and the layer-1 self path below.
            if skip_collective:
                nc.sync.dma_start(out=x_rep[0:OWN, :], in_=x_own[0:OWN, :])
            else:
                nc.sync.dma_start(out=x_stage[:, :], in_=x_own[0:OWN, :])
                nc.gpsimd.collective_compute(
                    "AllGather",
                    mybir.AluOpType.bypass,
                    replica_groups=[list(range(NCORES))],
                    ins=[x_stage[:, :]],
                    outs=[x_rep[0:N, :]],
                )

            rows_regs = {}

            def rows_reg(v):
                if v not in rows_regs:
                    rows_regs[v] = nc.gpsimd.to_reg(v)
                return rows_regs[v]

            iota = cpool.tile([P, P], dt.float32)
            nc.sync.dma_start(out=iota[:],
                              in_=iota_in.ap().rearrange("(p f) -> p f", p=P))
            ident = cpool.tile([P, P], dt.float32)
            make_identity(nc, ident[:])
            ident_bf = cpool.tile([P, P], dt.bfloat16)
            nc.vector.tensor_copy(out=ident_bf[:], in_=ident[:])

            wtiles = {}
            for name, t in (("ws1", ws1), ("wn1", wn1), ("ws2", ws2),
                            ("wn2", wn2)):
                wt = cpool.tile([P, P], dt.float32, name=f"w_{name}", tag=f"w_{name}")
                nc.sync.dma_start(out=wt[:], in_=t[:, :])
                wtiles[name] = wt
            btiles = {}
            for name, t in (("b1", b1), ("b2", b2)):
                bt = cpool.tile([P, 1], dt.float32, name=f"b_{name}", tag=f"b_{name}")
                nc.sync.dma_start(out=bt[:], in_=t.ap()[:, None])
                btiles[name] = bt

            gidx_t = cpool.tile([P, gidx_len // P], dt.int16)
            nc.sync.dma_start(out=gidx_t[:],
                              in_=gidx.ap().rearrange("(p k) -> p k", p=P))
            dstl_t = cpool.tile([P, nblk], dt.float32)
            nc.sync.dma_start(out=dstl_t[:],
                              in_=dstl_in.ap().rearrange("(p b) -> p b", p=P))
            w_t = cpool.tile([P, nblk], dt.float32)
            nc.sync.dma_start(out=w_t[:],
                              in_=w_in.ap().rearrange("(p b) -> p b", p=P))

            aggT = apool.tile([P, NTILES * P], dt.float32)
            selfT = apool.tile([P, NTILES * P], dt.float32)

            def run_layer(table, self_table, wself, wneigh, bias,
                          dest, tdt, ddt, identt):
                nc.vector.memset(aggT[:], 0.0)

                live_psum = {}
                blk_cursor = 0      # global block index
                gcol = 0            # idx columns consumed in gidx_t
                pass_blk = 0        # block index within current pass
                cur_pass = 0
                ginst = 0           # gather instruction counter
                for pp, take in inst_sizes:
                    if pp != cur_pass:
                        cur_pass = pp
                        pass_blk = 0
                    rows = take * BLK
                    icols = rows // 16
                    gt = gpool.tile([P, (GBS // BLK) * P], tdt, tag="g")
                    if no_gather:   # timing diagnostic: contiguous same-size DMA
                        for k in range(take):
                            nc.sync.dma_start(
                                out=gt[:, k * P : (k + 1) * P],
                                in_=table[pp * CHUNK + k * P
                                          : pp * CHUNK + (k + 1) * P, :])
                    else:
                        g = nc.gpsimd.dma_gather(
                            gt[:, : take * P].rearrange("p (b f) -> p b f", f=P),
                            table[pp * CHUNK : pp * CHUNK + pass_len[pp], :],
                            gidx_t[:, gcol : gcol + icols],
                            rows,
                            rows_reg(rows),
                            F,
                            queue_num=ginst % 4,
                        )
                        add_dep_helper(g.ins, lib.ins, sync=False,
                                       reason="ucode lib before gather")
                    gcol += icols
                    ginst += 1

                    for k in range(take if not no_agg else 0):
                        b = blk_cursor + k
                        t = int(blk_tile[b])
                        sel = spool.tile([P, P], tdt, tag="sel")
                        nc.vector.tensor_scalar(
                            sel[:], iota[:],
                            dstl_t[:, b : b + 1], w_t[:, b : b + 1],
                            mybir.AluOpType.is_equal, mybir.AluOpType.mult,
                        )
                        if firsts[pp][pass_blk + k]:
                            live_psum[t] = ppoolA.tile([P, P], dt.float32, name="ps",
                                                       tag="ps", space="PSUM")
                        ps = live_psum[t]
                        nc.tensor.matmul(
                            out=ps[:], lhsT=gt[:, k * P : (k + 1) * P],
                            rhs=sel[:],
                            start=bool(firsts[pp][pass_blk + k]),
                            stop=bool(lasts[pp][pass_blk + k]),
                        )
                        if lasts[pp][pass_blk + k]:
                            nc.vector.tensor_tensor(
                                out=aggT[:, t * P : (t + 1) * P],
                                in0=aggT[:, t * P : (t + 1) * P],
                                in1=ps[:], op=mybir.AluOpType.add,
                            )
                            del live_psum[t]
                    blk_cursor += take
                    pass_blk += take

                for t in range(NTILES):
                    xt = stpool.tile([P, P], tdt, tag="xt")
                    nc.sync.dma_start(out=xt[:],
                                      in_=self_table[t * P : (t + 1) * P, :])
                    pst = ppoolB.tile([P, P], tdt, tag="pst", space="PSUM")
                    nc.tensor.transpose(out=pst[:], in_=xt[:],
                                        identity=identt[:])
                    nc.vector.tensor_copy(out=selfT[:, t * P : (t + 1) * P],
                                          in_=pst[:])

                writes = []
                for g0 in range(0, NTILES, 4):
                    tn = min(4, NTILES - g0)
                    wdt = tn * P
                    psT = ppoolB.tile([P, 512], dt.float32, tag="psT",
                                      space="PSUM")
                    nc.tensor.matmul(out=psT[:, :wdt], lhsT=wneigh[:],
                                     rhs=aggT[:, g0 * P : g0 * P + wdt],
                                     start=True, stop=False)
                    nc.tensor.matmul(out=psT[:, :wdt], lhsT=wself[:],
                                     rhs=selfT[:, g0 * P : g0 * P + wdt],
                                     start=False, stop=True)
                    oT = stpool.tile([P, 512], dt.float32, tag="oT")
                    nc.scalar.activation(oT[:, :wdt], psT[:, :wdt],
                                         mybir.ActivationFunctionType.Relu,
                                         bias=bias[:, :1])
                    ost = stpool.tile([P, 512], ddt, tag="ost")
                    for j in range(tn):
                        psX = ppoolA.tile([P, P], dt.float32, tag="psX",
                                          space="PSUM")
                        nc.tensor.transpose(out=psX[:],
                                            in_=oT[:, j * P : (j + 1) * P],
                                            identity=ident[:])
                        nc.vector.tensor_copy(
                            out=ost[:, j * P : (j + 1) * P], in_=psX[:])
                    dd = nc.sync.dma_start(
                        out=dest[g0 * P : g0 * P + wdt, :]
                        .rearrange("(j p) f -> p j f", p=P),
                        in_=ost[:, :wdt].rearrange("p (j f) -> p j f", f=P),
                    )
                    writes.append(dd)
                return writes

            run_layer(x_rep, x_own, wtiles["ws1"], wtiles["wn1"],
                      btiles["b1"], h1_own, dt.bfloat16, dt.bfloat16,
                      ident_bf)

            if skip_collective:
                nc.sync.dma_start(out=h1_rep[0:OWN, :], in_=h1_own[0:OWN, :])
            else:
                nc.gpsimd.collective_compute(
                    "AllGather",
                    mybir.AluOpType.bypass,
                    replica_groups=[list(range(NCORES))],
                    ins=[h1_own[0:OWN, :]],
                    outs=[h1_rep[0:N, :]],
                )
            if N_PAD > N:
                zt = stpool.tile([P, F], dt.bfloat16, tag="zt")
                nc.vector.memset(zt[:], 0.0)
                nc.sync.dma_start(out=h1_rep[N:N_PAD, :],
                                  in_=zt[: N_PAD - N, :])

            run_layer(h1_rep, h1_own, wtiles["ws2"], wtiles["wn2"],
                      btiles["b2"], out_shard, dt.bfloat16, dt.float32,
                      ident_bf)

    _split_multi_waits(nc)
    from concourse.library_overlay import lower_extended_insts
    lower_extended_insts(nc)
    return nc


def _split_multi_waits(nc):
    pass_impl = True
    """Walrus codegen encodes at most one sync wait per instruction; split
    extras into standalone EventSemaphore instructions on the same in-order
    engine queue (semantically identical)."""
    import concourse.mybir as mybir

    n = 0
    for f in nc.m.functions:
        for b in f.blocks:
            insts = b.instructions
            new_list = []
            for inst in insts:
                si = inst.sync_info
                if si is not None and len(si.on_wait) > 1:
                    waits = list(si.on_wait)
                    for wt in waits[:-1]:
                        ev = mybir.InstEventSemaphore(
                            name=f"evsplit-{n}",
                            engine=inst.engine,
                            sync_info=mybir.SyncInfo(on_wait=[wt],
                                                     on_update=[]),
                            ins=[], outs=[],
                        )
                        new_list.append(ev)
                        try:
                            nc.inst_map[ev.name] = ev
                        except Exception:
                            pass
                        n += 1
                    inst.sync_info = mybir.SyncInfo(
                        on_wait=[waits[-1]], on_update=list(si.on_update)
                    )
                new_list.append(inst)
            insts[:] = new_list
    return n


# --------------------------------------------------------------------------
# entry point
# --------------------------------------------------------------------------

def prepare(x, edge_src, edge_dst, W_self1, W_neigh1, b1, W_self2, W_neigh2,
            b2):
    """Build the Bass program + per-core input maps (no execution)."""
    x = np.asarray(x, dtype=np.float32)
    plans, B, blk_tile, nblk_pass, nblk = _plan(edge_src, edge_dst)

    import ml_dtypes

    bf16 = ml_dtypes.bfloat16
    iota = np.broadcast_to(np.arange(P, dtype=np.float32), (P, P))

    in_maps = []
    for c in range(NCORES):
        pl = plans[c]
        xs = np.zeros((OWN_PAD, F), dtype=bf16)
        xs[:OWN] = x[c * OWN : (c + 1) * OWN].astype(bf16)
        in_maps.append({
            "x_own": xs,
            "gidx": _pack_gidx(pl["idx16"], nblk_pass),
            "dstl": pl["dstl"].reshape(nblk, P).T.copy().ravel(),
            "w": pl["w"].reshape(nblk, P).T.copy().ravel(),
            "iota": np.ascontiguousarray(iota).ravel(),
            "W_self1": np.asarray(W_self1, np.float32),
            "W_neigh1": np.asarray(W_neigh1, np.float32),
            "b1": np.asarray(b1, np.float32),
            "W_self2": np.asarray(W_self2, np.float32),
            "W_neigh2": np.asarray(W_neigh2, np.float32),
            "b2": np.asarray(b2, np.float32),
        })

    nc = _build(B, blk_tile, nblk_pass, nblk)
    return nc, in_maps


def assemble(results):
    return np.concatenate(
        [results[c]["out_shard"][:OWN] for c in range(NCORES)], axis=0
    ).astype(np.float32)


def kernel(x, edge_src, edge_dst, W_self1, W_neigh1, b1, W_self2, W_neigh2,
           b2, trace=False, _return_res=False):
    from concourse.bass_utils import run_bass_kernel_spmd

    nc, in_maps = prepare(x, edge_src, edge_dst, W_self1, W_neigh1, b1,
                          W_self2, W_neigh2, b2)
    res = run_bass_kernel_spmd(nc, in_maps, list(range(NCORES)), trace=trace)
    out = assemble(res.results)
    if _return_res:
        return out, res
    return out

